# revision 10
# baseline (speedup 1.0000x reference)
"""Trainium2 Bass kernel for a dense transformer block (nn_Block_65987877535901).

Collective-free sequence-parallel sharding over 8 NeuronCores:
core c handles query tokens [512*(c%4), 512*(c%4+1)) of batch c//4; K/V are
computed for the full batch on every core (replicated within the 4-core
batch group); causal masking via a host-uploaded multiplicative mask.

Main optimization vs the naive structure: LayerNorm is never materialized
on the input side.  LN is a per-token affine map, so it is folded PAST the
QKV / FC matmuls:
  LN(x)@W = rstd*(x@W' - mu*colsum(W')) + (b1@W + bias),  W' = diag(g)*W
The big matmuls therefore run on RAW x immediately (no vector-engine
prologue), with stats (mean/rstd) computed concurrently and applied as a
cheap 2-op PSUM correction.  For V the correction is folded into the
attention y-matmul itself: two extra stationary columns (mu_k, std_k) next
to the raw V, an rstd_k scale fused into the existing mask multiply, and
the constant term folded into the proj bias (host side).  K's additive
bias cancels in softmax and is dropped entirely.

All per-token stats processing happens in DMA-transposed [128, ntiles]
space so the vector engine always runs with full partition parallelism
(no 1-lane [1,512] reciprocals), with rstd computed as exp(-0.5*ln(var))
so the whole kernel needs only the natural_log_exp activation table (plus
one switch to gelu at the end).
"""
import sys

sys.path.insert(0, "/opt/trn_rl_repo")

import numpy as np
import ml_dtypes

BF16 = ml_dtypes.bfloat16

P = 128
C = 1024          # embed dim -> 8 chunks
NCH = C // P      # 8
T = 2048          # tokens per batch (kv length)
NT = 512          # own query tokens per core
H = 16            # heads
D = 64            # head dim
DA = D + 2        # V columns per head: [v(64) | mu | std]
F = 4096          # mlp hidden -> 32 chunks
NFCH = F // P     # 32
NKT = T // P      # 16 kv tiles
NCORES = 8
EPS = 1e-5

_COMPILED = None


def _build():
    import concourse.bacc as bacc
    import concourse.tile as tile
    import concourse.bass as bass
    from concourse import mybir

    dt = mybir.dt
    BF = dt.bfloat16
    F32 = dt.float32
    AF = mybir.ActivationFunctionType
    OP = mybir.AluOpType

    nc = bacc.Bacc("TRN2", target_bir_lowering=False, debug=False,
                   num_devices=NCORES)

    # ---- kernel I/O ----
    xkv = nc.declare_dram_parameter("xkv", [C, T], BF, isOutput=False)
    xq = nc.declare_dram_parameter("xq", [C, NT], BF, isOutput=False)
    xresb = nc.declare_dram_parameter("xresb", [C, NT], BF, isOutput=False)
    maskd = nc.declare_dram_parameter("maskd", [NKT, P, NT], BF, isOutput=False)
    wq = nc.declare_dram_parameter("wq", [C, C], BF, isOutput=False)
    wk = nc.declare_dram_parameter("wk", [C, C], BF, isOutput=False)
    wv = nc.declare_dram_parameter("wv", [C, C], BF, isOutput=False)
    wp = nc.declare_dram_parameter("wp", [C, C], BF, isOutput=False)
    wfc = nc.declare_dram_parameter("wfc", [C, F], BF, isOutput=False)
    wfc2 = nc.declare_dram_parameter("wfc2", [F, C], BF, isOutput=False)
    csqn = nc.declare_dram_parameter("csqn", [C], F32, isOutput=False)
    cskn = nc.declare_dram_parameter("cskn", [C], F32, isOutput=False)
    csvn = nc.declare_dram_parameter("csvn", [C], F32, isOutput=False)
    csfcn = nc.declare_dram_parameter("csfcn", [F], F32, isOutput=False)
    bq8 = nc.declare_dram_parameter("bq8", [C], F32, isOutput=False)
    bp = nc.declare_dram_parameter("bp", [C], F32, isOutput=False)
    bfc = nc.declare_dram_parameter("bfc", [F], F32, isOutput=False)
    bfc2 = nc.declare_dram_parameter("bfc2", [C], F32, isOutput=False)
    out = nc.declare_dram_parameter("out", [C, NT], F32, isOutput=True)

    def view(ap, doff, dims):
        """re-view an AP with an element offset delta and explicit dims"""
        return bass.AP(tensor=ap.tensor, offset=ap.offset + doff, ap=dims)

    def col_ap(dram, nparts, ncols):
        """[nparts*ncols] dram vector viewed [nparts, ncols] partition-major"""
        a = dram.ap()
        return view(a, 0, [[1, nparts], [nparts, ncols]])

    with (
        tile.TileContext(nc) as tc,
        tc.tile_pool(name="const", bufs=1) as const,
        tc.tile_pool(name="pBCD", bufs=1) as pBCD,
        tc.tile_pool(name="wpp", bufs=1) as wpp,
        tc.tile_pool(name="dramp", bufs=2, space="DRAM") as dramp,
    ):
        ones1 = const.tile([P, 1], BF, tag="ones1", name="ones1")
        nc.vector.memset(ones1[:], 1.0)
        eps128 = const.tile([P, 1], F32, tag="eps128", name="eps128")
        nc.vector.memset(eps128[:], EPS)

        csqn_t = const.tile([P, NCH], F32, tag="csqn", name="csqn")
        nc.sync.dma_start(csqn_t[:], col_ap(csqn, P, NCH))
        cskn_t = const.tile([P, NCH], F32, tag="cskn", name="cskn")
        nc.sync.dma_start(cskn_t[:], col_ap(cskn, P, NCH))
        csvn_t = const.tile([D, H], F32, tag="csvn", name="csvn")
        nc.sync.dma_start(csvn_t[:], col_ap(csvn, D, H))
        csfcn_t = const.tile([P, NFCH], F32, tag="csfcn", name="csfcn")
        nc.sync.dma_start(csfcn_t[:], col_ap(csfcn, P, NFCH))
        bq8_t = const.tile([P, NCH], F32, tag="bq8", name="bq8")
        nc.sync.dma_start(bq8_t[:], col_ap(bq8, P, NCH))
        bp_t = const.tile([P, NCH], F32, tag="bp", name="bp")
        nc.sync.dma_start(bp_t[:], col_ap(bp, P, NCH))
        bfc_t = const.tile([P, NFCH], F32, tag="bfc", name="bfc")
        nc.sync.dma_start(bfc_t[:], col_ap(bfc, P, NFCH))
        bfc2_t = const.tile([P, NCH], F32, tag="bfc2", name="bfc2")
        nc.sync.dma_start(bfc2_t[:], col_ap(bfc2, P, NCH))
        # per-key rstd in partition-major [128, NKT] space (used in B)
        rstd_kv = const.tile([P, NKT], F32, tag="rstdkv", name="rstdkv")

        # persistent across B..D
        yf = [pBCD.tile([P, NT], BF, tag=f"yf{m}", name=f"yf{m}")
              for m in range(NCH)]
        h1 = [pBCD.tile([P, NT], BF, tag=f"h1{m}", name=f"h1{m}")
              for m in range(NCH)]

        with tc.tile_pool(name="pAB", bufs=1) as pAB:
            # persistent across A..B
            kf = [pAB.tile([P, T], BF, tag=f"kf{m}", name=f"kf{m}")
                  for m in range(NCH)]
            vaug = [pAB.tile([P, H * DA], BF, tag=f"va{t}", name=f"va{t}")
                    for t in range(NKT)]
            qf = [pAB.tile([P, NT], BF, tag=f"qf{m}", name=f"qf{m}")
                  for m in range(NCH)]

            # ======== phase A: stats + QKV on raw x, LN folded ========
            with (
                tc.tile_pool(name="xkp", bufs=1) as xkp,
                tc.tile_pool(name="bcastp", bufs=1) as bcastp,
            ):
                xk = [xkp.tile([P, T], BF, tag=f"xk{i}", name=f"xk{i}")
                      for i in range(NCH)]
                for i in range(NCH):
                    nc.sync.dma_start(xk[i][:], xkv[i * P:(i + 1) * P, :])
                xqt = [xkp.tile([P, NT], BF, tag=f"xq{i}", name=f"xq{i}")
                       for i in range(NCH)]
                for i in range(NCH):
                    nc.sync.dma_start(xqt[i][:], xq[i * P:(i + 1) * P, :])
                # broadcast tiles for K/Q corrections (bf16)
                R_bc = [bcastp.tile([P, NT], BF, tag=f"Rbc{s}", name=f"Rbc{s}")
                        for s in range(4)]
                M_bc = [bcastp.tile([P, NT], BF, tag=f"Mbc{s}", name=f"Mbc{s}")
                        for s in range(4)]
                Rq_bc = bcastp.tile([P, NT], BF, tag="Rqbc", name="Rqbc")
                Mq_bc = bcastp.tile([P, NT], BF, tag="Mqbc", name="Mqbc")

                # ---- A1: stats + V ----
                with (
                    tc.tile_pool(name="wvp", bufs=1) as wvp,
                    tc.tile_pool(name="sqp", bufs=2) as sqp,
                    tc.tile_pool(name="statp", bufs=1) as statp,
                    tc.tile_pool(name="stgp", bufs=2) as stgp,
                    tc.tile_pool(name="psA1", bufs=4, space="PSUM") as psA1,
                    tc.tile_pool(name="psst", bufs=1, space="PSUM") as psst,
                ):
                    wvt = [wvp.tile([P, C], BF, tag=f"wv{k}", name=f"wv{k}")
                           for k in range(NCH)]
                    for k in range(NCH):
                        nc.sync.dma_start(wvt[k][:], wv[k * P:(k + 1) * P, :])

                    st_kv = psst.tile([P, 1024], F32, tag="stkv", name="stkv")
                    st_q = psst.tile([1, 1024], F32, tag="stq", name="stq")
                    # kv stats: sums and sumsq per 512-slab, rows at 32*s
                    for i in range(NCH):
                        sq = sqp.tile([P, T], BF, tag="sqk", name="sqk")
                        nc.scalar.activation(sq[:], xk[i][:], AF.Square)
                        for s in range(4):
                            nc.tensor.matmul(
                                st_kv[32 * s:32 * s + 1, 0:512], ones1[:],
                                xk[i][:, s * NT:(s + 1) * NT],
                                start=(i == 0), stop=(i == NCH - 1),
                                tile_position=(0, 32 * s))
                            nc.tensor.matmul(
                                st_kv[32 * s:32 * s + 1, 512:1024], ones1[:],
                                sq[:, s * NT:(s + 1) * NT],
                                start=(i == 0), stop=(i == NCH - 1),
                                tile_position=(0, 32 * s))
                    # q stats
                    for i in range(NCH):
                        sqq = sqp.tile([P, NT], BF, tag="sqq", name="sqq")
                        nc.scalar.activation(sqq[:], xqt[i][:], AF.Square)
                        nc.tensor.matmul(st_q[0:1, 0:512], ones1[:], xqt[i][:],
                                         start=(i == 0), stop=(i == NCH - 1))
                        nc.tensor.matmul(st_q[0:1, 512:1024], ones1[:], sqq[:],
                                         start=(i == 0), stop=(i == NCH - 1))

                    # ---- V matmuls (independent of stats) ----
                    for t in range(NKT):
                        for hh in range(2):
                            psv = psA1.tile([P, 512], F32, tag="psv",
                                            name="psv")
                            for k in range(NCH):
                                nc.tensor.matmul(
                                    psv[:], xk[k][:, t * P:(t + 1) * P],
                                    wvt[k][:, hh * 512:(hh + 1) * 512],
                                    start=(k == 0), stop=(k == NCH - 1))
                            # copy raw V into head-interleaved vaug slots
                            dst = view(vaug[t][:], hh * 8 * DA,
                                       [list(vaug[t][:].ap[0]), [DA, 8],
                                        [1, D]])
                            nc.scalar.activation(
                                dst, psv[:].rearrange("p (h x) -> p h x", h=8),
                                AF.Copy)

                    # ---- stats rows -> transposed space -> chain ----
                    scr_kv = dramp.tile([1, 4096], F32, tag="scrkv",
                                        name="scrkv")
                    scr_q = dramp.tile([1, 1024], F32, tag="scrq", name="scrq")
                    for s in range(4):
                        stg = stgp.tile([1, 1024], F32, tag="stg", name="stg")
                        nc.scalar.activation(stg[:],
                                             st_kv[32 * s:32 * s + 1, :],
                                             AF.Copy)
                        nc.sync.dma_start(
                            view(scr_kv[:], 1024 * s, [[1024, 1], [1, 1024]]),
                            stg[:])
                    stgq = stgp.tile([1, 1024], F32, tag="stg", name="stgq")
                    nc.scalar.activation(stgq[:], st_q[0:1, :], AF.Copy)
                    nc.sync.dma_start(scr_q[:], stgq[:])

                    # read back transposed: [128, 16] (kv) / [128, 4] (q)
                    sums = statp.tile([P, NKT], F32, tag="sums", name="sums")
                    sumsq = statp.tile([P, NKT], F32, tag="sumsq",
                                       name="sumsq")
                    for a in range(4):
                        nc.sync.dma_start(
                            sums[:, 4 * a:4 * a + 4],
                            view(scr_kv[:], 1024 * a, [[1, P], [128, 4]]))
                        nc.sync.dma_start(
                            sumsq[:, 4 * a:4 * a + 4],
                            view(scr_kv[:], 1024 * a + 512,
                                 [[1, P], [128, 4]]))
                    sums_q = statp.tile([P, 4], F32, tag="sumsg", name="sumsg")
                    sumsq_q = statp.tile([P, 4], F32, tag="sumsqg",
                                         name="sumsqg")
                    nc.sync.dma_start(sums_q[:],
                                      view(scr_q[:], 0, [[1, P], [P, 4]]))
                    nc.sync.dma_start(sumsq_q[:],
                                      view(scr_q[:], 512, [[1, P], [P, 4]]))

                    def stats_chain(s_pm, ss_pm, ncols, rstd_dst, tagp):
                        mu = statp.tile([P, ncols], F32, tag=f"mu{tagp}",
                                        name=f"mu{tagp}")
                        nc.vector.tensor_scalar_mul(mu[:], s_pm[:], 1.0 / C)
                        ex2 = statp.tile([P, ncols], F32, tag=f"ex{tagp}",
                                         name=f"ex{tagp}")
                        nc.vector.tensor_scalar_mul(ex2[:], ss_pm[:], 1.0 / C)
                        var = statp.tile([P, ncols], F32, tag=f"va{tagp}",
                                         name=f"va{tagp}")
                        nc.vector.tensor_mul(var[:], mu[:], mu[:])
                        nc.vector.tensor_sub(var[:], ex2[:], var[:])
                        lnv = statp.tile([P, ncols], F32, tag=f"ln{tagp}",
                                         name=f"ln{tagp}")
                        nc.scalar.activation(lnv[:], var[:], AF.Ln,
                                             bias=eps128[:])
                        nc.scalar.activation(rstd_dst, lnv[:], AF.Exp,
                                             scale=-0.5)
                        mmu = statp.tile([P, ncols], F32, tag=f"mm{tagp}",
                                        name=f"mm{tagp}")
                        nc.vector.tensor_mul(mmu[:], mu[:], rstd_dst)
                        return mu, lnv, mmu

                    mu_kv, lnv_kv, mmu_kv = stats_chain(
                        sums, sumsq, NKT, rstd_kv[:], "kv")
                    std_kv = statp.tile([P, NKT], F32, tag="stdkv",
                                        name="stdkv")
                    nc.scalar.activation(std_kv[:], lnv_kv[:], AF.Exp,
                                         scale=0.5)
                    rstd_q = statp.tile([P, 4], F32, tag="rstdq", name="rstdq")
                    mu_q, _lnv_q, mmu_q = stats_chain(
                        sums_q, sumsq_q, 4, rstd_q[:], "q")

                    # bf16 copies of mu/std for the vaug columns
                    musd_b = statp.tile([P, 2 * NKT], BF, tag="musdb",
                                        name="musdb")
                    nc.vector.tensor_copy(musd_b[:, 0:NKT], mu_kv[:])
                    nc.vector.tensor_copy(musd_b[:, NKT:2 * NKT], std_kv[:])
                    scr_musd = dramp.tile([1, 2 * T], BF, tag="scrmusd",
                                          name="scrmusd")
                    nc.sync.dma_start(
                        view(scr_musd[:], 0, [[1, P], [P, 2 * NKT]]),
                        musd_b[:])

                    # bf16 rstd/mmu rows -> DRAM -> slab broadcasts
                    rm_b = statp.tile([P, 2 * NKT], BF, tag="rmb", name="rmb")
                    nc.vector.tensor_copy(rm_b[:, 0:NKT], rstd_kv[:])
                    nc.vector.tensor_copy(rm_b[:, NKT:2 * NKT], mmu_kv[:])
                    scr_rm = dramp.tile([1, 2 * T], BF, tag="scrrm",
                                        name="scrrm")
                    nc.sync.dma_start(
                        view(scr_rm[:], 0, [[1, P], [P, 2 * NKT]]), rm_b[:])
                    rmq_b = statp.tile([P, 8], BF, tag="rmqb", name="rmqb")
                    nc.vector.tensor_copy(rmq_b[:, 0:4], rstd_q[:])
                    nc.vector.tensor_copy(rmq_b[:, 4:8], mmu_q[:])
                    scr_rmq = dramp.tile([1, 1024], BF, tag="scrrmq",
                                         name="scrrmq")
                    nc.sync.dma_start(
                        view(scr_rmq[:], 0, [[1, P], [P, 8]]), rmq_b[:])

                    for s in range(4):
                        nc.sync.dma_start(
                            R_bc[s][:],
                            view(scr_rm[:], NT * s, [[0, P], [1, NT]]))
                        nc.sync.dma_start(
                            M_bc[s][:],
                            view(scr_rm[:], T + NT * s, [[0, P], [1, NT]]))
                    nc.sync.dma_start(Rq_bc[:],
                                      view(scr_rmq[:], 0, [[0, P], [1, NT]]))
                    nc.sync.dma_start(Mq_bc[:],
                                      view(scr_rmq[:], NT, [[0, P], [1, NT]]))

                    # mu/std columns into vaug (per kv tile, all heads)
                    for t in range(NKT):
                        p0 = list(vaug[t][:].ap[0])
                        nc.sync.dma_start(
                            view(vaug[t][:], D, [p0, [DA, H]]),
                            view(scr_musd[:], P * t, [[1, P], [0, H]]))
                        nc.sync.dma_start(
                            view(vaug[t][:], D + 1, [p0, [DA, H]]),
                            view(scr_musd[:], T + P * t, [[1, P], [0, H]]))

                # ---- A2q: Q matmuls + corrections ----
                with (
                    tc.tile_pool(name="wqp", bufs=1) as wqp,
                    tc.tile_pool(name="tmpq", bufs=2) as tmpq,
                    tc.tile_pool(name="psQ", bufs=4, space="PSUM") as psQ,
                ):
                    wqt = [wqp.tile([P, C], BF, tag=f"wq{k}", name=f"wq{k}")
                           for k in range(NCH)]
                    for k in range(NCH):
                        nc.sync.dma_start(wqt[k][:], wq[k * P:(k + 1) * P, :])
                    for m in range(NCH):
                        ps = psQ.tile([P, NT], F32, tag="psq", name="psq")
                        for k in range(NCH):
                            nc.tensor.matmul(
                                ps[:], wqt[k][:, m * P:(m + 1) * P],
                                xqt[k][:], start=(k == 0),
                                stop=(k == NCH - 1))
                        t1 = tmpq.tile([P, NT], F32, tag="t1", name="t1")
                        nc.vector.scalar_tensor_tensor(
                            t1[:], Mq_bc[:], csqn_t[:, m:m + 1], ps[:],
                            OP.mult, OP.add)
                        t2 = tmpq.tile([P, NT], F32, tag="t2", name="t2")
                        nc.vector.scalar_tensor_tensor(
                            t2[:], t1[:], 0.0, Rq_bc[:], OP.bypass, OP.mult)
                        nc.vector.tensor_scalar_add(
                            qf[m][:], t2[:], bq8_t[:, m:m + 1])

                # ---- A2k: K matmuls + corrections ----
                with (
                    tc.tile_pool(name="wkp", bufs=1) as wkp,
                    tc.tile_pool(name="tmpk", bufs=2) as tmpk,
                    tc.tile_pool(name="psK", bufs=4, space="PSUM") as psK,
                ):
                    wkt = [wkp.tile([P, C], BF, tag=f"wk{k}", name=f"wk{k}")
                           for k in range(NCH)]
                    for k in range(NCH):
                        nc.sync.dma_start(wkt[k][:], wk[k * P:(k + 1) * P, :])
                    for m in range(NCH):
                        for s in range(4):
                            ps = psK.tile([P, NT], F32, tag="psk", name="psk")
                            for k in range(NCH):
                                nc.tensor.matmul(
                                    ps[:], wkt[k][:, m * P:(m + 1) * P],
                                    xk[k][:, s * NT:(s + 1) * NT],
                                    start=(k == 0), stop=(k == NCH - 1))
                            t1 = tmpk.tile([P, NT], F32, tag="t1", name="t1")
                            nc.vector.scalar_tensor_tensor(
                                t1[:], M_bc[s][:], cskn_t[:, m:m + 1], ps[:],
                                OP.mult, OP.add)
                            nc.vector.scalar_tensor_tensor(
                                kf[m][:, s * NT:(s + 1) * NT], t1[:], 0.0,
                                R_bc[s][:], OP.bypass, OP.mult)

            # ======== phase B: attention ========
            # prefetch proj weights (wpp, outlives pAB) + residual into h1
            wpt = [wpp.tile([P, C], BF, tag=f"wp{k}", name=f"wp{k}")
                   for k in range(NCH)]
            for k in range(NCH):
                nc.sync.dma_start(wpt[k][:], wp[k * P:(k + 1) * P, :])
            for m in range(NCH):
                nc.sync.dma_start(h1[m][:], xresb[m * P:(m + 1) * P, :])

            with (
                tc.tile_pool(name="maskp", bufs=1) as maskp,
                tc.tile_pool(name="epool", bufs=3) as epool,
                tc.tile_pool(name="ypost", bufs=2) as ypost,
                tc.tile_pool(name="ybc", bufs=2) as ybc,
                tc.tile_pool(name="pse", bufs=3, space="PSUM") as pse,
                tc.tile_pool(name="psy", bufs=1, space="PSUM") as psyp,
            ):
                maskt = maskp.tile([P, NKT * NT], BF, tag="mask", name="mask")
                nc.sync.dma_start(
                    maskt[:].rearrange("p (a b) -> p a b", a=NKT),
                    view(maskd.ap(), 0, [[NT, P], [P * NT, NKT], [1, NT]]))

                for hp in range(NCH):
                    psy = [psyp.tile([DA, NT], F32, tag=f"psy{u}",
                                     name=f"psy{u}") for u in range(2)]
                    for kg in range(8):
                        pss = [pse.tile([P, 2 * NT], F32, tag="pse",
                                        name="pse") for _ in range(2)]
                        for u in range(2):
                            for j in range(2):
                                kt = 2 * kg + j
                                nc.tensor.matmul(
                                    pss[u][:, j * NT:(j + 1) * NT],
                                    kf[hp][u * D:(u + 1) * D,
                                           kt * P:(kt + 1) * P],
                                    qf[hp][u * D:(u + 1) * D, :],
                                    start=True, stop=True)
                        for u in range(2):
                            etr = epool.tile([P, 2 * NT], BF, tag="etr",
                                             name="etr")
                            nc.scalar.activation(etr[:], pss[u][:], AF.Exp)
                            et = epool.tile([P, 2 * NT], BF, tag="et",
                                            name="et")
                            for j in range(2):
                                kt = 2 * kg + j
                                nc.vector.scalar_tensor_tensor(
                                    et[:, j * NT:(j + 1) * NT],
                                    etr[:, j * NT:(j + 1) * NT],
                                    rstd_kv[:, kt:kt + 1],
                                    maskt[:, kt * NT:(kt + 1) * NT],
                                    OP.mult, OP.mult)
                            for j in range(2):
                                kt = 2 * kg + j
                                h = 2 * hp + u
                                nc.tensor.matmul(
                                    psy[u][:],
                                    vaug[kt][:, h * DA:(h + 1) * DA],
                                    et[:, j * NT:(j + 1) * NT],
                                    start=(kg == 0 and j == 0),
                                    stop=(kg == 7 and j == 1))
                    # ---- per-head-pair normalization ----
                    ycop = [ypost.tile([DA, NT], F32, tag=f"yc{u}",
                                       name=f"yc{u}") for u in range(2)]
                    for u in range(2):
                        nc.scalar.activation(ycop[u][:], psy[u][:], AF.Copy)
                    scr_y = dramp.tile([1, 4 * NT], F32, tag="scry",
                                       name="scry")
                    for u in range(2):
                        nc.sync.dma_start(
                            view(scr_y[:], 2 * NT * u, [[NT, 2], [1, NT]]),
                            ycop[u][D:D + 2, :])
                    yst = ypost.tile([P, 16], F32, tag="yst", name="yst")
                    for u in range(2):
                        nc.sync.dma_start(
                            yst[:, 8 * u:8 * u + 4],
                            view(scr_y[:], 2 * NT * u, [[1, P], [P, 4]]))
                        nc.sync.dma_start(
                            yst[:, 8 * u + 4:8 * u + 8],
                            view(scr_y[:], 2 * NT * u + NT,
                                 [[1, P], [P, 4]]))
                    rw = ypost.tile([P, 16], F32, tag="rw", name="rw")
                    for u in range(2):
                        o = 8 * u
                        nc.vector.reciprocal(rw[:, o + 4:o + 8],
                                             yst[:, o + 4:o + 8])
                        nc.vector.tensor_mul(rw[:, o:o + 4], yst[:, o:o + 4],
                                             rw[:, o + 4:o + 8])
                    scr_rw = dramp.tile([1, 4 * NT], F32, tag="scrrw",
                                        name="scrrw")
                    nc.sync.dma_start(
                        view(scr_rw[:], 0, [[1, P], [P, 16]]), rw[:])
                    for u in range(2):
                        o = 8 * u
                        rbc = ybc.tile([D, NT], F32, tag="rbc", name="rbc")
                        nc.sync.dma_start(
                            rbc[:],
                            view(scr_rw[:], (o + 4) * P, [[0, D], [1, NT]]))
                        wbc = ybc.tile([D, NT], F32, tag="wbc", name="wbc")
                        nc.sync.dma_start(
                            wbc[:], view(scr_rw[:], o * P, [[0, D], [1, NT]]))
                        m1 = ybc.tile([D, NT], F32, tag="m1", name="m1")
                        nc.vector.tensor_mul(m1[:], ycop[u][0:D, :], rbc[:])
                        h = 2 * hp + u
                        nc.vector.scalar_tensor_tensor(
                            yf[hp][u * D:(u + 1) * D, :], wbc[:],
                            csvn_t[:, h:h + 1], m1[:], OP.mult, OP.add)

        # ======== phase C: proj + residual + LN2 stats ========
        with (
            tc.tile_pool(name="sq2p", bufs=2) as sq2p,
            tc.tile_pool(name="st2p", bufs=1) as st2p,
            tc.tile_pool(name="bc2p", bufs=1) as bc2p,
        ):
            stg2 = st2p.tile([1, 1024], F32, tag="stg2", name="stg2")
            with (
                tc.tile_pool(name="ps4", bufs=4, space="PSUM") as ps4,
                tc.tile_pool(name="psst2", bufs=1, space="PSUM") as psst2,
            ):
                st2 = psst2.tile([1, 1024], F32, tag="st2", name="st2")
                for m in range(NCH):
                    ps = ps4.tile([P, NT], F32, tag="pj", name="pj")
                    for k in range(NCH):
                        nc.tensor.matmul(
                            ps[:], wpt[k][:, m * P:(m + 1) * P], yf[k][:],
                            start=(k == 0), stop=(k == NCH - 1))
                    nc.vector.scalar_tensor_tensor(
                        h1[m][:], ps[:], bp_t[:, m:m + 1], h1[m][:],
                        OP.add, OP.add)
                    sq2 = sq2p.tile([P, NT], BF, tag="sq2", name="sq2")
                    nc.scalar.activation(sq2[:], h1[m][:], AF.Square)
                    nc.tensor.matmul(st2[0:1, 0:512], ones1[:], h1[m][:],
                                     start=(m == 0), stop=(m == NCH - 1))
                    nc.tensor.matmul(st2[0:1, 512:1024], ones1[:], sq2[:],
                                     start=(m == 0), stop=(m == NCH - 1))
                # LN2 stats rows out of PSUM before the pools close
                nc.scalar.activation(stg2[:], st2[0:1, :], AF.Copy)
            scr2 = dramp.tile([1, 1024], F32, tag="scr2", name="scr2")
            nc.sync.dma_start(scr2[:], stg2[:])
            s2 = st2p.tile([P, 4], F32, tag="s2", name="s2")
            ss2 = st2p.tile([P, 4], F32, tag="ss2", name="ss2")
            nc.sync.dma_start(s2[:], view(scr2[:], 0, [[1, P], [P, 4]]))
            nc.sync.dma_start(ss2[:], view(scr2[:], 512, [[1, P], [P, 4]]))
            mu2 = st2p.tile([P, 4], F32, tag="mu2", name="mu2")
            nc.vector.tensor_scalar_mul(mu2[:], s2[:], 1.0 / C)
            ex22 = st2p.tile([P, 4], F32, tag="ex22", name="ex22")
            nc.vector.tensor_scalar_mul(ex22[:], ss2[:], 1.0 / C)
            var2 = st2p.tile([P, 4], F32, tag="var2", name="var2")
            nc.vector.tensor_mul(var2[:], mu2[:], mu2[:])
            nc.vector.tensor_sub(var2[:], ex22[:], var2[:])
            lnv2 = st2p.tile([P, 4], F32, tag="lnv2", name="lnv2")
            nc.scalar.activation(lnv2[:], var2[:], AF.Ln, bias=eps128[:])
            rstd2 = st2p.tile([P, 4], F32, tag="rstd2", name="rstd2")
            nc.scalar.activation(rstd2[:], lnv2[:], AF.Exp, scale=-0.5)
            mmu2 = st2p.tile([P, 4], F32, tag="mmu2", name="mmu2")
            nc.vector.tensor_mul(mmu2[:], mu2[:], rstd2[:])
            rm2_b = st2p.tile([P, 8], BF, tag="rm2b", name="rm2b")
            nc.vector.tensor_copy(rm2_b[:, 0:4], rstd2[:])
            nc.vector.tensor_copy(rm2_b[:, 4:8], mmu2[:])
            scr_rm2 = dramp.tile([1, 1024], BF, tag="scrrm2", name="scrrm2")
            nc.sync.dma_start(
                view(scr_rm2[:], 0, [[1, P], [P, 8]]), rm2_b[:])
            R2_bc = bc2p.tile([P, NT], BF, tag="R2bc", name="R2bc")
            nc.sync.dma_start(R2_bc[:],
                              view(scr_rm2[:], 0, [[0, P], [1, NT]]))
            M2_bc = bc2p.tile([P, NT], BF, tag="M2bc", name="M2bc")
            nc.sync.dma_start(M2_bc[:],
                              view(scr_rm2[:], NT, [[0, P], [1, NT]]))

            # ======== phase D: MLP ========
            with tc.tile_pool(name="hmlpp", bufs=1) as hmlpp:
                hmlp = [hmlpp.tile([P, NT], BF, tag=f"hm{m}", name=f"hm{m}")
                        for m in range(NFCH)]
                with (
                    tc.tile_pool(name="wfcp", bufs=1) as wfcp,
                    tc.tile_pool(name="tmpf", bufs=4) as tmpf,
                    tc.tile_pool(name="ps6", bufs=4, space="PSUM") as ps6,
                ):
                    wfct = [wfcp.tile([P, F], BF, tag=f"wfc{k}",
                                      name=f"wfc{k}") for k in range(NCH)]
                    for k in range(NCH):
                        nc.sync.dma_start(wfct[k][:],
                                          wfc[k * P:(k + 1) * P, :])
                    for mg in range(NFCH // 4):
                        pss = [ps6.tile([P, NT], F32, tag="fc", name="fc")
                               for _ in range(4)]
                        for k in range(NCH):
                            for mm in range(4):
                                m = 4 * mg + mm
                                nc.tensor.matmul(
                                    pss[mm][:],
                                    wfct[k][:, m * P:(m + 1) * P],
                                    h1[k][:], start=(k == 0),
                                    stop=(k == NCH - 1))
                        for mm in range(4):
                            m = 4 * mg + mm
                            t1 = tmpf.tile([P, NT], F32, tag="t1", name="t1")
                            nc.vector.scalar_tensor_tensor(
                                t1[:], M2_bc[:], csfcn_t[:, m:m + 1],
                                pss[mm][:], OP.mult, OP.add)
                            t2 = tmpf.tile([P, NT], BF, tag="t2", name="t2")
                            nc.vector.scalar_tensor_tensor(
                                t2[:], t1[:], 0.0, R2_bc[:],
                                OP.bypass, OP.mult)
                            nc.scalar.activation(
                                hmlp[m][:], t2[:], AF.Gelu,
                                bias=bfc_t[:, m:m + 1])
                # fc2 + residual + store
                with (
                    tc.tile_pool(name="wfc2p", bufs=3) as wfc2p,
                    tc.tile_pool(name="ps7", bufs=1, space="PSUM") as ps7,
                    tc.tile_pool(name="outp", bufs=2) as outp,
                ):
                    pso = [ps7.tile([P, NT], F32, tag=f"fo{m}",
                                    name=f"fo{m}") for m in range(NCH)]
                    for k in range(NFCH):
                        wt2 = wfc2p.tile([P, C], BF, tag="wfc2", name="wfc2")
                        nc.sync.dma_start(wt2[:], wfc2[k * P:(k + 1) * P, :])
                        for m in range(NCH):
                            nc.tensor.matmul(
                                pso[m][:], wt2[:, m * P:(m + 1) * P],
                                hmlp[k][:], start=(k == 0),
                                stop=(k == NFCH - 1))
                    for m in range(NCH):
                        ot = outp.tile([P, NT], F32, tag="ot", name="ot")
                        nc.vector.scalar_tensor_tensor(
                            ot[:], pso[m][:], bfc2_t[:, m:m + 1], h1[m][:],
                            OP.add, OP.add)
                        nc.sync.dma_start(out[m * P:(m + 1) * P, :], ot[:])

    nc.compile()
    return nc


def _host_prep(x, ln1_g, ln1_b, W_attn, b_attn, W_proj, b_proj,
               ln2_g, ln2_b, W_fc, b_fc, W_fc2, b_fc2):
    """Build the 8 per-core input maps (LN folds precomputed in fp32)."""
    x = np.asarray(x, dtype=np.float32)
    W_attn = np.asarray(W_attn, dtype=np.float32)
    b_attn = np.asarray(b_attn, dtype=np.float32)
    g1 = np.asarray(ln1_g, np.float32)
    b1 = np.asarray(ln1_b, np.float32)
    g2 = np.asarray(ln2_g, np.float32)
    b2 = np.asarray(ln2_b, np.float32)
    Wp = np.asarray(W_proj, np.float32)
    Wfc = np.asarray(W_fc, np.float32)
    Wfc2 = np.asarray(W_fc2, np.float32)

    Wq = W_attn[:, 0:C]
    Wk = W_attn[:, C:2 * C]
    Wv = W_attn[:, 2 * C:3 * C]
    # folded weights: W' = diag(g1) @ W  (Q additionally pre-scaled by 1/8)
    wq_f = (g1[:, None] * Wq) / 8.0
    wk_f = g1[:, None] * Wk
    wv_f = g1[:, None] * Wv
    wfc_f = g2[:, None] * Wfc
    # negated column sums for the (u - mu*colsum)*rstd corrections
    csqn = -wq_f.sum(0)
    cskn = -wk_f.sum(0)
    csvn = -wv_f.sum(0)
    csfcn = -wfc_f.sum(0)
    # additive constants: b1@W + bias.  K's constant cancels in softmax.
    bq8 = (b1 @ Wq + b_attn[0:C]) / 8.0
    v_const = b1 @ Wv + b_attn[2 * C:3 * C]
    bp_eff = np.asarray(b_proj, np.float32) + v_const @ Wp
    bfc_eff = b2 @ Wfc + np.asarray(b_fc, np.float32)

    shared = dict(
        wq=wq_f.astype(BF16), wk=wk_f.astype(BF16), wv=wv_f.astype(BF16),
        wp=Wp.astype(BF16), wfc=wfc_f.astype(BF16), wfc2=Wfc2.astype(BF16),
        csqn=csqn.astype(np.float32), cskn=cskn.astype(np.float32),
        csvn=csvn.astype(np.float32), csfcn=csfcn.astype(np.float32),
        bq8=bq8.astype(np.float32), bp=bp_eff.astype(np.float32),
        bfc=bfc_eff.astype(np.float32),
        bfc2=np.asarray(b_fc2, np.float32),
    )

    # causal masks per slab position s: mask[kt, k, t] = (kt*128+k <= 512s+t)
    kpos = np.arange(T).reshape(NKT, P, 1)
    tpos = np.arange(NT).reshape(1, 1, NT)
    masks = [(kpos <= 512 * s + tpos).astype(BF16) for s in range(4)]

    in_maps = []
    for c in range(NCORES):
        b, s = c // 4, c % 4
        xb = x[b]                      # [T, C]
        xo = xb[512 * s:512 * (s + 1)]  # [NT, C]
        m = dict(shared)
        m["xkv"] = np.ascontiguousarray(xb.T).astype(BF16)
        m["xq"] = np.ascontiguousarray(xo.T).astype(BF16)
        m["xresb"] = np.ascontiguousarray(xo.T).astype(BF16)
        m["maskd"] = masks[s]
        in_maps.append(m)
    return in_maps


def kernel(x, ln1_g, ln1_b, W_attn, b_attn, W_proj, b_proj,
           ln2_g, ln2_b, W_fc, b_fc, W_fc2, b_fc2):
    global _COMPILED
    from concourse.bass_utils import run_bass_kernel_spmd

    if _COMPILED is None:
        _COMPILED = _build()
    nc = _COMPILED
    in_maps = _host_prep(x, ln1_g, ln1_b, W_attn, b_attn, W_proj, b_proj,
                         ln2_g, ln2_b, W_fc, b_fc, W_fc2, b_fc2)
    res = run_bass_kernel_spmd(nc, in_maps, list(range(NCORES)))
    out = np.empty((2, T, C), dtype=np.float32)
    for c in range(NCORES):
        b, s = c // 4, c % 4
        out[b, 512 * s:512 * (s + 1), :] = res.results[c]["out"].T
    return out
